# revision 27
# baseline (speedup 1.0000x reference)
"""Trainium2 Bass kernel for CTRMultiEmbedding (embedding_lookup).

Math (per batch b, with EX_SU=1, EX_SL=0, EX_TU=1, EX_TL=0):
  joint[p,:]   = emb_t[t_idx[p]] + emb_l[l[p]] + emb_u[u[p]]          [256, 64]
  m            = (i < len) & (j < len)                       in {0,1}
  delta[i,j,:] = emb_sl[m]*(1-ds) + emb_su[m]*ds
               + emb_tl[m]*(1-dt) + emb_tu[m]*dt                      [256,256,64]

delta is a per-pair linear combination of 8 raw table rows with scalar
coefficients [(1-m)(1-ds), m(1-ds), (1-m)ds, m ds, (1-m)(1-dt), m(1-dt),
(1-m)dt, m dt].  We compute it as a matmul: for each tile of 1024 pairs,
an 8-block block-diagonal weight matrix W [64, 512] (8 features x 8 pair
blocks; raw table rows on the diagonal blocks) is multiplied by a feature
matrix lhsT [64, 128] so that the PSUM result [128, 512] is *exactly* the
contiguous DRAM layout of 1024 output rows.

Sharding: pure data parallel, batch b -> core b (8 cores, B=8).
"""

import numpy as np

import concourse.bass as bass
import concourse.mybir as mybir
from concourse import bacc
from concourse.bass_utils import run_bass_kernel_spmd
from concourse.masks import make_identity
from concourse.tile import TileContext

B, L, D = 8, 256, 64
NPAIR = L * L            # 65536 pairs per batch
G = 8                    # pair-blocks per matmul tile
NF = 8                   # features per block
K = G * NF               # 64 contraction rows
TILE_PAIRS = 128 * G     # 1024 pairs per tile
NT = NPAIR // TILE_PAIRS  # 64 tiles
NFREE = G * D            # 512 moving columns

F32 = mybir.dt.float32
F32R = mybir.dt.float32r
I32 = mybir.dt.int32


def build_program():
    # Bacc (not raw Bass): its compile step legalizes multi-wait sync into
    # separate wait ops — walrus allows only one sem wait per instruction.
    nc = bacc.Bacc("TRN2", target_bir_lowering=False, debug=False)

    mat = nc.dram_tensor("mat", [NPAIR, 2], F32, kind="ExternalInput")
    idx6 = nc.dram_tensor("idx6", [128, 6], I32, kind="ExternalInput")
    len1 = nc.dram_tensor("len1", [1, 1], I32, kind="ExternalInput")
    wmat = nc.dram_tensor("wmat", [K, NFREE], F32, kind="ExternalInput")
    emb_t = nc.dram_tensor("emb_t", [169, D], F32, kind="ExternalInput")
    emb_l = nc.dram_tensor("emb_l", [50000, D], F32, kind="ExternalInput")
    emb_u = nc.dram_tensor("emb_u", [10000, D], F32, kind="ExternalInput")
    joint = nc.dram_tensor("joint", [L, D], F32, kind="ExternalOutput")
    delta = nc.dram_tensor("delta", [NPAIR, D], F32, kind="ExternalOutput")

    with TileContext(nc) as tc:
        with (
            tc.tile_pool(name="const", bufs=1) as cpool,
            tc.tile_pool(name="big", bufs=1) as bigpool,
            tc.tile_pool(name="lhs", bufs=4) as lhspool,
            tc.tile_pool(name="stg", bufs=NT) as stgpool,
            tc.tile_pool(name="outp", bufs=6) as outpool,
            tc.tile_pool(name="tpp", bufs=2, space="PSUM") as tppool,
            tc.tile_pool(name="mmp", bufs=4, space="PSUM") as mmpool,
        ):
            # ---------------- one-time setup ----------------
            identity = cpool.tile([128, 128], F32)
            make_identity(nc, identity[:])

            w_f32 = cpool.tile([K, NFREE], F32)
            nc.sync.dma_start(out=w_f32[:], in_=wmat[:])
            w_sb = cpool.tile([K, NFREE], F32R)
            nc.vector.tensor_copy(out=w_sb[:], in_=w_f32[:])

            # mat_all[m, t*16 + g*2 + c] = mat[1024t + 8m + g, c]
            mat_all = bigpool.tile([128, NT * 16], F32)
            mat_src = mat[:].rearrange("(t m g) c -> m t g c", t=NT, m=128, g=G)
            nc.sync.dma_start(
                out=mat_all[:].rearrange("p (t g c) -> p t g c", t=NT, g=G, c=2),
                in_=mat_src,
            )

            len_col = cpool.tile([128, 1], I32)
            nc.sync.dma_start(out=len_col[:], in_=len1[:].to_broadcast([128, 1]))
            len_f = cpool.tile([128, 1], F32)
            nc.vector.tensor_copy(out=len_f[:], in_=len_col[:])

            idx_sb = cpool.tile([128, 6], I32)
            nc.sync.dma_start(out=idx_sb[:], in_=idx6[:])

            # ---------------- index / mask precompute ----------------
            # pair q = 1024t + 8m + g;  j = q & 255;  i = 4t + (m >> 5)
            jg = cpool.tile([128, G], I32)
            nc.gpsimd.iota(jg[:], pattern=[[1, G]], base=0, channel_multiplier=8)
            nc.vector.tensor_scalar(
                out=jg[:], in0=jg[:], scalar1=255, scalar2=None,
                op0=mybir.AluOpType.bitwise_and,
            )
            jgf = cpool.tile([128, G], F32)
            nc.vector.tensor_copy(out=jgf[:], in_=jg[:])
            jl = cpool.tile([128, G], F32)  # (j < len) as 1.0/0.0
            nc.vector.tensor_scalar(
                out=jl[:], in0=jgf[:], scalar1=len_f[:, 0:1], scalar2=None,
                op0=mybir.AluOpType.is_lt,
            )

            mcol = cpool.tile([128, 1], I32)
            nc.gpsimd.iota(mcol[:], pattern=[[1, 1]], base=0, channel_multiplier=1)
            nc.vector.tensor_scalar(
                out=mcol[:], in0=mcol[:], scalar1=5, scalar2=None,
                op0=mybir.AluOpType.arith_shift_right,
            )
            mcolf = cpool.tile([128, 1], F32)
            nc.vector.tensor_copy(out=mcolf[:], in_=mcol[:])
            ii = cpool.tile([128, NT], I32)
            nc.gpsimd.iota(ii[:], pattern=[[4, NT]], base=0, channel_multiplier=0)
            iif = cpool.tile([128, NT], F32)
            nc.vector.tensor_copy(out=iif[:], in_=ii[:])
            nc.vector.tensor_scalar(
                out=iif[:], in0=iif[:], scalar1=mcolf[:, 0:1], scalar2=None,
                op0=mybir.AluOpType.add,
            )
            vi = cpool.tile([128, NT], F32)  # (i < len) as 1.0/0.0
            nc.vector.tensor_scalar(
                out=vi[:], in0=iif[:], scalar1=len_f[:, 0:1], scalar2=None,
                op0=mybir.AluOpType.is_lt,
            )

            # Expand vi over g and jl over t to flat [128, NT*G] (t-major)
            # buffers so every big op below is 1-free-dim (S2S2D2 encodings).
            NTG = NT * G
            vix = bigpool.tile([128, NTG], F32)
            nc.vector.tensor_copy(
                out=vix[:],
                in_=vi[:].unsqueeze(2).to_broadcast([128, NT, G]),
            )
            jlx = bigpool.tile([128, NTG], F32)
            nc.vector.tensor_copy(
                out=jlx[:],
                in_=jl[:].unsqueeze(1).to_broadcast([128, NT, G]),
            )
            mask = bigpool.tile([128, NTG], F32)  # vi * jl
            nc.vector.tensor_tensor(
                out=mask[:], in0=vix[:], in1=jlx[:], op=mybir.AluOpType.mult,
            )
            mbar = bigpool.tile([128, NTG], F32)  # 1 - mask
            nc.vector.tensor_scalar(
                out=mbar[:], in0=mask[:], scalar1=-1.0, scalar2=1.0,
                op0=mybir.AluOpType.mult, op1=mybir.AluOpType.add,
            )

            mat4 = mat_all[:].rearrange("p (t g c) -> p t g c", t=NT, g=G, c=2)
            ds = bigpool.tile([128, NTG], F32)
            nc.vector.tensor_copy(out=ds[:], in_=mat4[:, :, :, 0])
            dt_ = bigpool.tile([128, NTG], F32)
            nc.vector.tensor_copy(out=dt_[:], in_=mat4[:, :, :, 1])
            dsbar = bigpool.tile([128, NTG], F32)  # 1 - ds
            nc.vector.tensor_scalar(
                out=dsbar[:], in0=ds[:], scalar1=-1.0, scalar2=1.0,
                op0=mybir.AluOpType.mult, op1=mybir.AluOpType.add,
            )
            dtbar = bigpool.tile([128, NTG], F32)  # 1 - dt
            nc.vector.tensor_scalar(
                out=dtbar[:], in0=dt_[:], scalar1=-1.0, scalar2=1.0,
                op0=mybir.AluOpType.mult, op1=mybir.AluOpType.add,
            )

            # ---------------- feature matrix F ----------------
            # f_all[m, c*512 + t*8 + g] = feature c of pair (t, m, g)
            f_all = bigpool.tile([128, NF * NTG], F32)
            feats = [
                (mbar, dsbar),   # c=0: (1-m)(1-ds) -> emb_sl[0]
                (mask, dsbar),   # c=1: m(1-ds)     -> emb_sl[1]
                (mbar, ds),      # c=2: (1-m)ds     -> emb_su[0]
                (mask, ds),      # c=3: m ds        -> emb_su[1]
                (mbar, dtbar),   # c=4: (1-m)(1-dt) -> emb_tl[0]
                (mask, dtbar),   # c=5: m(1-dt)     -> emb_tl[1]
                (mbar, dt_),     # c=6: (1-m)dt     -> emb_tu[0]
                (mask, dt_),     # c=7: m dt        -> emb_tu[1]
            ]
            for c, (a, bb) in enumerate(feats):
                nc.vector.tensor_tensor(
                    out=f_all[:, c * NTG:(c + 1) * NTG],
                    in0=a[:], in1=bb[:], op=mybir.AluOpType.mult,
                )
            # per-tile gather view: [128, (c, g)] for tile t at [:, :, t, :]
            f5 = f_all[:].rearrange("p (c t g) -> p c t g", c=NF, t=NT, g=G)

            # ---------------- joint embedding (gathers) ----------------
            # t_idx = ((t + 167) mod 168) + 1  (t + 167 == t - 1 mod 168, >= 0)
            # No mod ALU op: binary conditional-subtraction in exact f32 ints.
            tf = cpool.tile([128, 2], F32)
            nc.vector.tensor_copy(out=tf[:], in_=idx_sb[:, 4:6])
            nc.vector.tensor_scalar(
                out=tf[:], in0=tf[:], scalar1=167.0, scalar2=None,
                op0=mybir.AluOpType.add,
            )
            tcorr = cpool.tile([128, 2], F32)
            for k in range(5, -1, -1):
                c = float(168 << k)
                # tcorr = (tf >= c) * (-c);  tf += tcorr
                nc.vector.tensor_scalar(
                    out=tcorr[:], in0=tf[:], scalar1=c, scalar2=-c,
                    op0=mybir.AluOpType.is_ge, op1=mybir.AluOpType.mult,
                )
                nc.vector.tensor_tensor(
                    out=tf[:], in0=tf[:], in1=tcorr[:], op=mybir.AluOpType.add,
                )
            nc.vector.tensor_scalar(
                out=tf[:], in0=tf[:], scalar1=1.0, scalar2=None,
                op0=mybir.AluOpType.add,
            )
            tix = cpool.tile([128, 2], I32)
            nc.vector.tensor_copy(out=tix[:], in_=tf[:])

            # Sum the three gathers with DMA inline-accumulate (CCE add):
            # no vector-engine ops needed for the joint embedding at all.
            js = cpool.tile([128, 2 * D], F32)
            gathers = [
                (emb_u, idx_sb[:, 0:1], 0, mybir.AluOpType.bypass),
                (emb_u, idx_sb[:, 1:2], 1, mybir.AluOpType.bypass),
                (emb_l, idx_sb[:, 2:3], 0, mybir.AluOpType.add),
                (emb_l, idx_sb[:, 3:4], 1, mybir.AluOpType.add),
                (emb_t, tix[:, 0:1], 0, mybir.AluOpType.add),
                (emb_t, tix[:, 1:2], 1, mybir.AluOpType.add),
            ]
            for table, iap, half, op in gathers:
                nc.gpsimd.indirect_dma_start(
                    out=js[:, half * D:(half + 1) * D], out_offset=None,
                    in_=table[:],
                    in_offset=bass.IndirectOffsetOnAxis(ap=iap, axis=0),
                    compute_op=op,
                )
            nc.sync.dma_start(
                out=joint[:].rearrange("(h p) d -> p h d", h=2),
                in_=js[:].rearrange("p (h d) -> p h d", h=2),
            )

            # ---------------- main loop: 64 tiles of 1024 pairs ----------------
            delta4 = delta[:].rearrange("(t m g) d -> t m g d", t=NT, m=128, g=G)
            for t in range(NT):
                stage = stgpool.tile([128, K], F32, tag="stage")
                nc.vector.tensor_copy(out=stage[:], in_=f5[:, :, t, :])
                tp = tppool.tile([K, 128], F32)
                nc.tensor.transpose(
                    out=tp[:], in_=stage[:], identity=identity[:],
                )
                lhsT = lhspool.tile([K, 128], F32R)
                nc.scalar.copy(out=lhsT[:], in_=tp[:])

                mm = mmpool.tile([128, NFREE], F32)
                nc.tensor.matmul(
                    out=mm[:],
                    lhsT=lhsT[:],
                    rhs=w_sb[:],
                    start=True, stop=True,
                )

                ot = outpool.tile([128, NFREE], F32)
                nc.scalar.copy(out=ot[:], in_=mm[:])
                nc.sync.dma_start(
                    out=delta4[t],
                    in_=ot[:].rearrange("p (g d) -> p g d", g=G),
                )
    nc.finalize()
    return nc


_NC_CACHE = {}


def _get_program():
    if "nc" not in _NC_CACHE:
        _NC_CACHE["nc"] = build_program()
    return _NC_CACHE["nc"]


def _make_in_maps(traj_input, mat_input, traj_length,
                  emb_t, emb_l, emb_u, emb_su, emb_sl, emb_tu, emb_tl):
    emb_t = np.ascontiguousarray(emb_t, dtype=np.float32)
    emb_l = np.ascontiguousarray(emb_l, dtype=np.float32)
    emb_u = np.ascontiguousarray(emb_u, dtype=np.float32)

    # Block-diagonal weights from raw table rows (no arithmetic, placement only).
    rows = [emb_sl[0], emb_sl[1], emb_su[0], emb_su[1],
            emb_tl[0], emb_tl[1], emb_tu[0], emb_tu[1]]
    # lhsT row r = 8c + g (transpose enumerates (c, g) row-major)
    wmat = np.zeros((K, NFREE), dtype=np.float32)
    for g in range(G):
        for c in range(NF):
            wmat[c * G + g, g * D:(g + 1) * D] = rows[c]

    in_maps = []
    for b in range(B):
        traj = np.asarray(traj_input[b], dtype=np.int32)   # [256, 3]
        idx6 = np.empty((128, 6), dtype=np.int32)
        idx6[:, 0] = traj[:128, 0]
        idx6[:, 1] = traj[128:, 0]
        idx6[:, 2] = traj[:128, 1]
        idx6[:, 3] = traj[128:, 1]
        idx6[:, 4] = traj[:128, 2]
        idx6[:, 5] = traj[128:, 2]
        in_maps.append({
            "mat": np.ascontiguousarray(
                np.asarray(mat_input[b], dtype=np.float32).reshape(NPAIR, 2)),
            "idx6": idx6,
            "len1": np.array([[traj_length[b]]], dtype=np.int32),
            "wmat": wmat,
            "emb_t": emb_t,
            "emb_l": emb_l,
            "emb_u": emb_u,
        })
    return in_maps


def run(trace=False, **inputs):
    nc = _get_program()
    in_maps = _make_in_maps(**inputs)
    res = run_bass_kernel_spmd(nc, in_maps, core_ids=list(range(B)), trace=trace)
    joint = np.stack([res.results[b]["joint"] for b in range(B)])
    delta = np.stack(
        [res.results[b]["delta"].reshape(L, L, D) for b in range(B)])
    return (joint, delta), res


def kernel(**inputs):
    out, _ = run(trace=False, **inputs)
    return out


# revision 38
# speedup vs baseline: 35039.6274x; 35039.6274x over previous
"""Trainium2 Bass kernel for CTRMultiEmbedding (embedding_lookup).

Math (per batch b, with EX_SU=1, EX_SL=0, EX_TU=1, EX_TL=0):
  joint[p,:]   = emb_t[t_idx[p]] + emb_l[l[p]] + emb_u[u[p]]          [256, 64]
  m            = (i < len) & (j < len)                       in {0,1}
  delta[i,j,:] = emb_sl[m]*(1-ds) + emb_su[m]*ds
               + emb_tl[m]*(1-dt) + emb_tu[m]*dt                      [256,256,64]

delta is a per-pair linear combination of 8 raw table rows with scalar
coefficients [(1-m)(1-ds), m(1-ds), (1-m)ds, m ds, (1-m)(1-dt), m(1-dt),
(1-m)dt, m dt].  We compute it as a matmul: for each tile of 1024 pairs,
an 8-block block-diagonal weight matrix W [64, 512] (8 features x 8 pair
blocks; raw table rows on the diagonal blocks) is multiplied by a feature
matrix lhsT [64, 128] so that the PSUM result [128, 512] is *exactly* the
contiguous DRAM layout of 1024 output rows.

Sharding: pure data parallel, batch b -> core b (8 cores, B=8).
"""

import numpy as np

import concourse.bass as bass
import concourse.mybir as mybir
from concourse import bacc
from concourse.bass_utils import run_bass_kernel_spmd
from concourse.masks import make_identity
from concourse.tile import TileContext

B, L, D = 8, 256, 64
NPAIR = L * L            # 65536 pairs per batch
G = 8                    # pair-blocks per matmul tile
NF = 8                   # features per block
K = G * NF               # 64 contraction rows
TILE_PAIRS = 128 * G     # 1024 pairs per tile
NT = NPAIR // TILE_PAIRS  # 64 tiles
NFREE = G * D            # 512 moving columns

F32 = mybir.dt.float32
F32R = mybir.dt.float32r
I32 = mybir.dt.int32


def build_program():
    # Bacc (not raw Bass): its compile step legalizes multi-wait sync into
    # separate wait ops — walrus allows only one sem wait per instruction.
    nc = bacc.Bacc("TRN2", target_bir_lowering=False, debug=False)

    mat = nc.dram_tensor("mat", [NPAIR, 2], F32, kind="ExternalInput")
    idx6 = nc.dram_tensor("idx6", [128, 6], I32, kind="ExternalInput")
    len1 = nc.dram_tensor("len1", [1, 1], I32, kind="ExternalInput")
    wmat = nc.dram_tensor("wmat", [K, NFREE], F32, kind="ExternalInput")
    emb_t = nc.dram_tensor("emb_t", [169, D], F32, kind="ExternalInput")
    emb_l = nc.dram_tensor("emb_l", [50000, D], F32, kind="ExternalInput")
    emb_u = nc.dram_tensor("emb_u", [10000, D], F32, kind="ExternalInput")
    joint = nc.dram_tensor("joint", [L, D], F32, kind="ExternalOutput")
    delta = nc.dram_tensor("delta", [NPAIR, D], F32, kind="ExternalOutput")

    with TileContext(nc) as tc:
        with (
            tc.tile_pool(name="const", bufs=1) as cpool,
            tc.tile_pool(name="big", bufs=1) as bigpool,
            tc.tile_pool(name="lhs", bufs=6) as lhspool,
            tc.tile_pool(name="outp", bufs=8) as outpool,
            tc.tile_pool(name="tpp", bufs=3, space="PSUM") as tppool,
            tc.tile_pool(name="mmp", bufs=5, space="PSUM") as mmpool,
        ):
            # ---------------- one-time setup ----------------
            identity = cpool.tile([128, 128], F32)
            make_identity(nc, identity[:])

            w_f32 = cpool.tile([K, NFREE], F32)
            nc.sync.dma_start(out=w_f32[:], in_=wmat[:])
            w_sb = cpool.tile([K, NFREE], F32R)
            nc.vector.tensor_copy(out=w_sb[:], in_=w_f32[:])

            # mat_all[m, t*16 + g*2 + c] = mat[1024t + 8m + g, c]
            # (loaded in NCHUNK pieces inside the main chunk loop below)
            mat_all = bigpool.tile([128, NT * 16], F32)
            mat_src = mat[:].rearrange("(t m g) c -> m t g c", t=NT, m=128, g=G)

            len_col = cpool.tile([128, 1], I32)
            nc.sync.dma_start(out=len_col[:], in_=len1[:].to_broadcast([128, 1]))
            len_f = cpool.tile([128, 1], F32)
            nc.vector.tensor_copy(out=len_f[:], in_=len_col[:])

            idx_sb = cpool.tile([128, 6], I32)
            nc.sync.dma_start(out=idx_sb[:], in_=idx6[:])

            # ---------------- index / mask precompute ----------------
            # pair q = 1024t + 8m + g;  j = q & 255;  i = 4t + (m >> 5)
            jg = cpool.tile([128, G], I32)
            nc.gpsimd.iota(jg[:], pattern=[[1, G]], base=0, channel_multiplier=8)
            nc.vector.tensor_scalar(
                out=jg[:], in0=jg[:], scalar1=255, scalar2=None,
                op0=mybir.AluOpType.bitwise_and,
            )
            jgf = cpool.tile([128, G], F32)
            nc.vector.tensor_copy(out=jgf[:], in_=jg[:])
            jl = cpool.tile([128, G], F32)  # (j < len) as 1.0/0.0
            nc.vector.tensor_scalar(
                out=jl[:], in0=jgf[:], scalar1=len_f[:, 0:1], scalar2=None,
                op0=mybir.AluOpType.is_lt,
            )

            mcol = cpool.tile([128, 1], I32)
            nc.gpsimd.iota(mcol[:], pattern=[[1, 1]], base=0, channel_multiplier=1)
            nc.vector.tensor_scalar(
                out=mcol[:], in0=mcol[:], scalar1=5, scalar2=None,
                op0=mybir.AluOpType.arith_shift_right,
            )
            mcolf = cpool.tile([128, 1], F32)
            nc.vector.tensor_copy(out=mcolf[:], in_=mcol[:])
            ii = cpool.tile([128, NT], I32)
            nc.gpsimd.iota(ii[:], pattern=[[4, NT]], base=0, channel_multiplier=0)
            iif = cpool.tile([128, NT], F32)
            nc.vector.tensor_copy(out=iif[:], in_=ii[:])
            nc.vector.tensor_scalar(
                out=iif[:], in0=iif[:], scalar1=mcolf[:, 0:1], scalar2=None,
                op0=mybir.AluOpType.add,
            )
            vi = cpool.tile([128, NT], F32)  # (i < len) as 1.0/0.0
            nc.vector.tensor_scalar(
                out=vi[:], in0=iif[:], scalar1=len_f[:, 0:1], scalar2=None,
                op0=mybir.AluOpType.is_lt,
            )

            # Expand vi over g and jl over t to flat [128, NT*G] (t-major)
            # buffers so every big op below is 1-free-dim (S2S2D2 encodings).
            NTG = NT * G
            vix = bigpool.tile([128, NTG], F32)
            nc.vector.tensor_copy(
                out=vix[:],
                in_=vi[:].unsqueeze(2).to_broadcast([128, NT, G]),
            )
            jlx = bigpool.tile([128, NTG], F32)
            nc.vector.tensor_copy(
                out=jlx[:],
                in_=jl[:].unsqueeze(1).to_broadcast([128, NT, G]),
            )
            mask = bigpool.tile([128, NTG], F32)  # vi * jl
            nc.vector.tensor_tensor(
                out=mask[:], in0=vix[:], in1=jlx[:], op=mybir.AluOpType.mult,
            )
            mbar = bigpool.tile([128, NTG], F32)  # 1 - mask
            nc.vector.tensor_scalar(
                out=mbar[:], in0=mask[:], scalar1=-1.0, scalar2=1.0,
                op0=mybir.AluOpType.mult, op1=mybir.AluOpType.add,
            )

            # ---------------- feature matrix F ----------------
            # f_all[m, t*64 + c*8 + g] = feature c of pair (t, m, g)
            # (t-major so each tile's stationary slice is contiguous)
            # Features: c2=mbar*ds c3=mask*ds c6=mbar*dt c7=mask*dt, then
            # c0=mbar-c2, c1=mask-c3, c4=mbar-c6, c5=mask-c7.
            # Built in NCHUNK t-chunks so matmuls can start early.
            f_all = bigpool.tile([128, NF * NTG], F32)
            f6 = f_all[:].rearrange("p (t c g) -> p t c g", t=NT, c=NF, g=G)
            mat4 = mat_all[:].rearrange("p (t g c) -> p t g c", t=NT, g=G, c=2)
            mask3 = mask[:].rearrange("p (t g) -> p t g", t=NT, g=G)
            mbar3 = mbar[:].rearrange("p (t g) -> p t g", t=NT, g=G)
            NCHUNK = 8
            TC = NT // NCHUNK

            delta4 = delta[:].rearrange("(t m g) d -> t m g d", t=NT, m=128, g=G)
            for q in range(NCHUNK):
                ts_ = slice(q * TC, (q + 1) * TC)
                # load this chunk of mat
                nc.sync.dma_start(
                    out=mat4[:, ts_], in_=mat_src[:, ts_],
                )
                msk, mbr = mask3[:, ts_, :], mbar3[:, ts_, :]
                ds = mat4[:, ts_, :, 0]
                dt_ = mat4[:, ts_, :, 1]
                prods = [(2, mbr, ds), (3, msk, ds), (6, mbr, dt_), (7, msk, dt_)]
                for c, a, bb in prods:
                    nc.vector.tensor_tensor(
                        out=f6[:, ts_, c, :], in0=a, in1=bb,
                        op=mybir.AluOpType.mult,
                    )
                diffs = [(0, mbr, 2), (1, msk, 3), (4, mbr, 6), (5, msk, 7)]
                for c, a, csrc in diffs:
                    nc.vector.tensor_tensor(
                        out=f6[:, ts_, c, :], in0=a, in1=f6[:, ts_, csrc, :],
                        op=mybir.AluOpType.subtract,
                    )

                # ---- 16 output tiles of this chunk ----
                for ti in range(TC):
                    t = q * TC + ti
                    tp = tppool.tile([K, 128], F32)
                    nc.tensor.transpose(
                        out=tp[:], in_=f_all[:, t * K:(t + 1) * K],
                        identity=identity[:],
                    )
                    lhsT = lhspool.tile([K, 128], F32R)
                    nc.scalar.copy(out=lhsT[:], in_=tp[:])

                    mm = mmpool.tile([128, NFREE], F32)
                    nc.tensor.matmul(
                        out=mm[:], lhsT=lhsT[:], rhs=w_sb[:],
                        start=True, stop=True,
                    )

                    ot = outpool.tile([128, NFREE], F32)
                    if t % 16 < 7:
                        nc.scalar.copy(out=ot[:], in_=mm[:])
                    else:
                        nc.vector.tensor_copy(out=ot[:], in_=mm[:])
                    dma_eng = nc.gpsimd if (t % 4 == 3 or t % 16 == 1) else nc.sync
                    dma_eng.dma_start(
                        out=delta4[t],
                        in_=ot[:].rearrange("p (g d) -> p g d", g=G),
                    )

            # ---------------- joint embedding (gathers) ----------------
            # t_idx = ((t + 167) mod 168) + 1  (t + 167 == t - 1 mod 168, >= 0)
            # No mod ALU op: binary conditional-subtraction in exact f32 ints.
            tf = cpool.tile([128, 2], F32)
            nc.vector.tensor_copy(out=tf[:], in_=idx_sb[:, 4:6])
            nc.vector.tensor_scalar(
                out=tf[:], in0=tf[:], scalar1=167.0, scalar2=None,
                op0=mybir.AluOpType.add,
            )
            tcorr = cpool.tile([128, 2], F32)
            for k in range(5, -1, -1):
                c = float(168 << k)
                # tcorr = (tf >= c) * (-c);  tf += tcorr
                nc.vector.tensor_scalar(
                    out=tcorr[:], in0=tf[:], scalar1=c, scalar2=-c,
                    op0=mybir.AluOpType.is_ge, op1=mybir.AluOpType.mult,
                )
                nc.vector.tensor_tensor(
                    out=tf[:], in0=tf[:], in1=tcorr[:], op=mybir.AluOpType.add,
                )
            nc.vector.tensor_scalar(
                out=tf[:], in0=tf[:], scalar1=1.0, scalar2=None,
                op0=mybir.AluOpType.add,
            )
            tix = cpool.tile([128, 2], I32)
            nc.vector.tensor_copy(out=tix[:], in_=tf[:])

            # Sum the three gathers with DMA inline-accumulate (CCE add):
            # no vector-engine ops needed for the joint embedding at all.
            js = cpool.tile([128, 2 * D], F32)
            gathers = [
                (emb_u, idx_sb[:, 0:1], 0, mybir.AluOpType.bypass),
                (emb_u, idx_sb[:, 1:2], 1, mybir.AluOpType.bypass),
                (emb_l, idx_sb[:, 2:3], 0, mybir.AluOpType.add),
                (emb_l, idx_sb[:, 3:4], 1, mybir.AluOpType.add),
                (emb_t, tix[:, 0:1], 0, mybir.AluOpType.add),
                (emb_t, tix[:, 1:2], 1, mybir.AluOpType.add),
            ]
            for table, iap, half, op in gathers:
                nc.gpsimd.indirect_dma_start(
                    out=js[:, half * D:(half + 1) * D], out_offset=None,
                    in_=table[:],
                    in_offset=bass.IndirectOffsetOnAxis(ap=iap, axis=0),
                    compute_op=op,
                )
            nc.sync.dma_start(
                out=joint[:].rearrange("(h p) d -> p h d", h=2),
                in_=js[:].rearrange("p (h d) -> p h d", h=2),
            )
    nc.finalize()
    return nc


_NC_CACHE = {}


def _get_program():
    if "nc" not in _NC_CACHE:
        _NC_CACHE["nc"] = build_program()
    return _NC_CACHE["nc"]


def _make_in_maps(traj_input, mat_input, traj_length,
                  emb_t, emb_l, emb_u, emb_su, emb_sl, emb_tu, emb_tl):
    emb_t = np.ascontiguousarray(emb_t, dtype=np.float32)
    emb_l = np.ascontiguousarray(emb_l, dtype=np.float32)
    emb_u = np.ascontiguousarray(emb_u, dtype=np.float32)

    # Block-diagonal weights from raw table rows (no arithmetic, placement only).
    rows = [emb_sl[0], emb_sl[1], emb_su[0], emb_su[1],
            emb_tl[0], emb_tl[1], emb_tu[0], emb_tu[1]]
    # lhsT row r = 8c + g (transpose enumerates (c, g) row-major)
    wmat = np.zeros((K, NFREE), dtype=np.float32)
    for g in range(G):
        for c in range(NF):
            wmat[c * G + g, g * D:(g + 1) * D] = rows[c]

    in_maps = []
    for b in range(B):
        traj = np.asarray(traj_input[b], dtype=np.int32)   # [256, 3]
        idx6 = np.empty((128, 6), dtype=np.int32)
        idx6[:, 0] = traj[:128, 0]
        idx6[:, 1] = traj[128:, 0]
        idx6[:, 2] = traj[:128, 1]
        idx6[:, 3] = traj[128:, 1]
        idx6[:, 4] = traj[:128, 2]
        idx6[:, 5] = traj[128:, 2]
        in_maps.append({
            "mat": np.ascontiguousarray(
                np.asarray(mat_input[b], dtype=np.float32).reshape(NPAIR, 2)),
            "idx6": idx6,
            "len1": np.array([[traj_length[b]]], dtype=np.int32),
            "wmat": wmat,
            "emb_t": emb_t,
            "emb_l": emb_l,
            "emb_u": emb_u,
        })
    return in_maps


def run(trace=False, **inputs):
    nc = _get_program()
    in_maps = _make_in_maps(**inputs)
    res = run_bass_kernel_spmd(nc, in_maps, core_ids=list(range(B)), trace=trace)
    joint = np.stack([res.results[b]["joint"] for b in range(B)])
    delta = np.stack(
        [res.results[b]["delta"].reshape(L, L, D) for b in range(B)])
    return (joint, delta), res


def kernel(**inputs):
    out, _ = run(trace=False, **inputs)
    return out


# revision 43
# speedup vs baseline: 36874.1321x; 1.0524x over previous
"""Trainium2 Bass kernel for CTRMultiEmbedding (embedding_lookup).

Math (per batch b, with EX_SU=1, EX_SL=0, EX_TU=1, EX_TL=0):
  joint[p,:]   = emb_t[t_idx[p]] + emb_l[l[p]] + emb_u[u[p]]          [256, 64]
  m            = (i < len) & (j < len)                       in {0,1}
  delta[i,j,:] = emb_sl[m]*(1-ds) + emb_su[m]*ds
               + emb_tl[m]*(1-dt) + emb_tu[m]*dt                      [256,256,64]

delta is a per-pair linear combination of 8 raw table rows with scalar
coefficients [(1-m)(1-ds), m(1-ds), (1-m)ds, m ds, (1-m)(1-dt), m(1-dt),
(1-m)dt, m dt].  We compute it as a matmul: for each tile of 1024 pairs,
an 8-block block-diagonal weight matrix W [64, 512] (8 features x 8 pair
blocks; raw table rows on the diagonal blocks) is multiplied by a feature
matrix lhsT [64, 128] so that the PSUM result [128, 512] is *exactly* the
contiguous DRAM layout of 1024 output rows.

Sharding: pure data parallel, batch b -> core b (8 cores, B=8).
"""

import numpy as np

import concourse.bass as bass
import concourse.mybir as mybir
from concourse import bacc
from concourse.bass_utils import run_bass_kernel_spmd
from concourse.masks import make_identity
from concourse.tile import TileContext

B, L, D = 8, 256, 64
NPAIR = L * L            # 65536 pairs per batch
G = 8                    # pair-blocks per matmul tile
NF = 8                   # features per block
K = G * NF               # 64 contraction rows
TILE_PAIRS = 128 * G     # 1024 pairs per tile
NT = NPAIR // TILE_PAIRS  # 64 tiles
NFREE = G * D            # 512 moving columns

F32 = mybir.dt.float32
F32R = mybir.dt.float32r
I32 = mybir.dt.int32


def build_program():
    # Bacc (not raw Bass): its compile step legalizes multi-wait sync into
    # separate wait ops — walrus allows only one sem wait per instruction.
    nc = bacc.Bacc("TRN2", target_bir_lowering=False, debug=False)

    mat = nc.dram_tensor("mat", [NPAIR, 2], F32, kind="ExternalInput")
    idx6 = nc.dram_tensor("idx6", [128, 6], I32, kind="ExternalInput")
    len1 = nc.dram_tensor("len1", [1, 1], I32, kind="ExternalInput")
    wmat = nc.dram_tensor("wmat", [K, NFREE], F32, kind="ExternalInput")
    emb_t = nc.dram_tensor("emb_t", [169, D], F32, kind="ExternalInput")
    emb_l = nc.dram_tensor("emb_l", [50000, D], F32, kind="ExternalInput")
    emb_u = nc.dram_tensor("emb_u", [10000, D], F32, kind="ExternalInput")
    joint = nc.dram_tensor("joint", [L, D], F32, kind="ExternalOutput")
    delta = nc.dram_tensor("delta", [NPAIR, D], F32, kind="ExternalOutput")

    with TileContext(nc) as tc:
        with (
            tc.tile_pool(name="const", bufs=1) as cpool,
            tc.tile_pool(name="big", bufs=1) as bigpool,
            tc.tile_pool(name="lhs", bufs=6) as lhspool,
            tc.tile_pool(name="outp", bufs=8) as outpool,
            tc.tile_pool(name="tpp", bufs=3, space="PSUM") as tppool,
            tc.tile_pool(name="mmp", bufs=5, space="PSUM") as mmpool,
        ):
            # ---------------- one-time setup ----------------
            identity = cpool.tile([128, 128], F32)
            make_identity(nc, identity[:])

            w_f32 = cpool.tile([K, NFREE], F32)
            nc.sync.dma_start(out=w_f32[:], in_=wmat[:])
            # W duplicated in both partition halves: the two matmuls per
            # transposed pair use lhsT at base partition 0 and 64, and
            # matmul requires rhs to share lhsT's base partition.
            w_sb = cpool.tile([128, NFREE], F32R)
            nc.scalar.copy(out=w_sb[0:K, :], in_=w_f32[:])
            nc.scalar.copy(out=w_sb[K:2 * K, :], in_=w_f32[:])

            # mat_all[m, t*16 + g*2 + c] = mat[1024t + 8m + g, c]
            # (loaded in NCHUNK pieces inside the main chunk loop below)
            mat_all = bigpool.tile([128, NT * 16], F32)
            mat_src = mat[:].rearrange("(t m g) c -> m t g c", t=NT, m=128, g=G)

            len_col = cpool.tile([128, 1], I32)
            nc.sync.dma_start(out=len_col[:], in_=len1[:].to_broadcast([128, 1]))
            len_f = cpool.tile([128, 1], F32)
            nc.vector.tensor_copy(out=len_f[:], in_=len_col[:])

            idx_sb = cpool.tile([128, 6], I32)
            nc.sync.dma_start(out=idx_sb[:], in_=idx6[:])

            # ---------------- index / mask precompute ----------------
            # pair q = 1024t + 8m + g;  j = q & 255;  i = 4t + (m >> 5)
            jg = cpool.tile([128, G], I32)
            nc.gpsimd.iota(jg[:], pattern=[[1, G]], base=0, channel_multiplier=8)
            nc.vector.tensor_scalar(
                out=jg[:], in0=jg[:], scalar1=255, scalar2=None,
                op0=mybir.AluOpType.bitwise_and,
            )
            jgf = cpool.tile([128, G], F32)
            nc.vector.tensor_copy(out=jgf[:], in_=jg[:])
            jl = cpool.tile([128, G], F32)  # (j < len) as 1.0/0.0
            nc.vector.tensor_scalar(
                out=jl[:], in0=jgf[:], scalar1=len_f[:, 0:1], scalar2=None,
                op0=mybir.AluOpType.is_lt,
            )

            mcol = cpool.tile([128, 1], I32)
            nc.gpsimd.iota(mcol[:], pattern=[[1, 1]], base=0, channel_multiplier=1)
            nc.vector.tensor_scalar(
                out=mcol[:], in0=mcol[:], scalar1=5, scalar2=None,
                op0=mybir.AluOpType.arith_shift_right,
            )
            mcolf = cpool.tile([128, 1], F32)
            nc.vector.tensor_copy(out=mcolf[:], in_=mcol[:])
            ii = cpool.tile([128, NT], I32)
            nc.gpsimd.iota(ii[:], pattern=[[4, NT]], base=0, channel_multiplier=0)
            iif = cpool.tile([128, NT], F32)
            nc.vector.tensor_copy(out=iif[:], in_=ii[:])
            nc.vector.tensor_scalar(
                out=iif[:], in0=iif[:], scalar1=mcolf[:, 0:1], scalar2=None,
                op0=mybir.AluOpType.add,
            )
            vi = cpool.tile([128, NT], F32)  # (i < len) as 1.0/0.0
            nc.vector.tensor_scalar(
                out=vi[:], in0=iif[:], scalar1=len_f[:, 0:1], scalar2=None,
                op0=mybir.AluOpType.is_lt,
            )

            # Expand vi over g and jl over t to flat [128, NT*G] (t-major)
            # buffers so every big op below is 1-free-dim (S2S2D2 encodings).
            NTG = NT * G
            vix = bigpool.tile([128, NTG], F32)
            nc.gpsimd.tensor_copy(
                out=vix[:],
                in_=vi[:].unsqueeze(2).to_broadcast([128, NT, G]),
            )
            jlx = bigpool.tile([128, NTG], F32)
            nc.gpsimd.tensor_copy(
                out=jlx[:],
                in_=jl[:].unsqueeze(1).to_broadcast([128, NT, G]),
            )
            mask = bigpool.tile([128, NTG], F32)  # vi * jl
            nc.vector.tensor_tensor(
                out=mask[:], in0=vix[:], in1=jlx[:], op=mybir.AluOpType.mult,
            )
            mbar = bigpool.tile([128, NTG], F32)  # 1 - mask
            nc.vector.tensor_scalar(
                out=mbar[:], in0=mask[:], scalar1=-1.0, scalar2=1.0,
                op0=mybir.AluOpType.mult, op1=mybir.AluOpType.add,
            )

            # ---------------- feature matrix F ----------------
            # f_all[m, t*64 + c*8 + g] = feature c of pair (t, m, g)
            # (t-major so each tile's stationary slice is contiguous)
            # Features: c2=mbar*ds c3=mask*ds c6=mbar*dt c7=mask*dt, then
            # c0=mbar-c2, c1=mask-c3, c4=mbar-c6, c5=mask-c7.
            # Built in NCHUNK t-chunks so matmuls can start early.
            f_all = bigpool.tile([128, NF * NTG], F32)
            f6 = f_all[:].rearrange("p (t c g) -> p t c g", t=NT, c=NF, g=G)
            mat4 = mat_all[:].rearrange("p (t g c) -> p t g c", t=NT, g=G, c=2)
            mask3 = mask[:].rearrange("p (t g) -> p t g", t=NT, g=G)
            mbar3 = mbar[:].rearrange("p (t g) -> p t g", t=NT, g=G)
            NCHUNK = 8
            TC = NT // NCHUNK

            delta4 = delta[:].rearrange("(t m g) d -> t m g d", t=NT, m=128, g=G)
            for q in range(NCHUNK):
                ts_ = slice(q * TC, (q + 1) * TC)
                # load this chunk of mat
                nc.sync.dma_start(
                    out=mat4[:, ts_], in_=mat_src[:, ts_],
                )
                msk, mbr = mask3[:, ts_, :], mbar3[:, ts_, :]
                ds = mat4[:, ts_, :, 0]
                dt_ = mat4[:, ts_, :, 1]
                prods = [(2, mbr, ds), (3, msk, ds), (6, mbr, dt_), (7, msk, dt_)]
                for c, a, bb in prods:
                    nc.vector.tensor_tensor(
                        out=f6[:, ts_, c, :], in0=a, in1=bb,
                        op=mybir.AluOpType.mult,
                    )
                diffs = [(0, mbr, 2), (1, msk, 3), (4, mbr, 6), (5, msk, 7)]
                for c, a, csrc in diffs:
                    nc.vector.tensor_tensor(
                        out=f6[:, ts_, c, :], in0=a, in1=f6[:, ts_, csrc, :],
                        op=mybir.AluOpType.subtract,
                    )

                # ---- 16 output tiles of this chunk (2 per transpose) ----
                for tp2 in range(TC // 2):
                    t0 = q * TC + 2 * tp2
                    tp = tppool.tile([128, 128], F32)
                    nc.tensor.transpose(
                        out=tp[:], in_=f_all[:, t0 * K:(t0 + 2) * K],
                        identity=identity[:],
                    )
                    lhsT = lhspool.tile([128, 128], F32R)
                    nc.scalar.copy(out=lhsT[:], in_=tp[:])

                    for half in range(2):
                        t = t0 + half
                        mm = mmpool.tile([128, NFREE], F32)
                        nc.tensor.matmul(
                            out=mm[:], lhsT=lhsT[half * K:(half + 1) * K, :],
                            rhs=w_sb[half * K:(half + 1) * K, :],
                            start=True, stop=True,
                        )

                        ot = outpool.tile([128, NFREE], F32)
                        if t % 16 < 9:
                            nc.scalar.copy(out=ot[:], in_=mm[:])
                        else:
                            nc.vector.tensor_copy(out=ot[:], in_=mm[:])
                        dma_eng = nc.gpsimd if (t % 4 == 3 or t % 16 in (1, 9)) else nc.sync
                        dma_eng.dma_start(
                            out=delta4[t],
                            in_=ot[:].rearrange("p (g d) -> p g d", g=G),
                        )

            # ---------------- joint embedding (gathers) ----------------
            # t_idx = ((t + 167) mod 168) + 1  (t + 167 == t - 1 mod 168, >= 0)
            # No mod ALU op: binary conditional-subtraction in exact f32 ints.
            tf = cpool.tile([128, 2], F32)
            nc.vector.tensor_copy(out=tf[:], in_=idx_sb[:, 4:6])
            nc.vector.tensor_scalar(
                out=tf[:], in0=tf[:], scalar1=167.0, scalar2=None,
                op0=mybir.AluOpType.add,
            )
            tcorr = cpool.tile([128, 2], F32)
            for k in range(5, -1, -1):
                c = float(168 << k)
                # tcorr = (tf >= c) * (-c);  tf += tcorr
                nc.vector.tensor_scalar(
                    out=tcorr[:], in0=tf[:], scalar1=c, scalar2=-c,
                    op0=mybir.AluOpType.is_ge, op1=mybir.AluOpType.mult,
                )
                nc.vector.tensor_tensor(
                    out=tf[:], in0=tf[:], in1=tcorr[:], op=mybir.AluOpType.add,
                )
            nc.vector.tensor_scalar(
                out=tf[:], in0=tf[:], scalar1=1.0, scalar2=None,
                op0=mybir.AluOpType.add,
            )
            tix = cpool.tile([128, 2], I32)
            nc.vector.tensor_copy(out=tix[:], in_=tf[:])

            # Sum the three gathers with DMA inline-accumulate (CCE add):
            # no vector-engine ops needed for the joint embedding at all.
            js = cpool.tile([128, 2 * D], F32)
            gathers = [
                (emb_u, idx_sb[:, 0:1], 0, mybir.AluOpType.bypass),
                (emb_u, idx_sb[:, 1:2], 1, mybir.AluOpType.bypass),
                (emb_l, idx_sb[:, 2:3], 0, mybir.AluOpType.add),
                (emb_l, idx_sb[:, 3:4], 1, mybir.AluOpType.add),
                (emb_t, tix[:, 0:1], 0, mybir.AluOpType.add),
                (emb_t, tix[:, 1:2], 1, mybir.AluOpType.add),
            ]
            for table, iap, half, op in gathers:
                nc.gpsimd.indirect_dma_start(
                    out=js[:, half * D:(half + 1) * D], out_offset=None,
                    in_=table[:],
                    in_offset=bass.IndirectOffsetOnAxis(ap=iap, axis=0),
                    compute_op=op,
                )
            nc.sync.dma_start(
                out=joint[:].rearrange("(h p) d -> p h d", h=2),
                in_=js[:].rearrange("p (h d) -> p h d", h=2),
            )
    nc.finalize()
    return nc


_NC_CACHE = {}


def _get_program():
    if "nc" not in _NC_CACHE:
        _NC_CACHE["nc"] = build_program()
    return _NC_CACHE["nc"]


def _make_in_maps(traj_input, mat_input, traj_length,
                  emb_t, emb_l, emb_u, emb_su, emb_sl, emb_tu, emb_tl):
    emb_t = np.ascontiguousarray(emb_t, dtype=np.float32)
    emb_l = np.ascontiguousarray(emb_l, dtype=np.float32)
    emb_u = np.ascontiguousarray(emb_u, dtype=np.float32)

    # Block-diagonal weights from raw table rows (no arithmetic, placement only).
    rows = [emb_sl[0], emb_sl[1], emb_su[0], emb_su[1],
            emb_tl[0], emb_tl[1], emb_tu[0], emb_tu[1]]
    # lhsT row r = 8c + g (transpose enumerates (c, g) row-major)
    wmat = np.zeros((K, NFREE), dtype=np.float32)
    for g in range(G):
        for c in range(NF):
            wmat[c * G + g, g * D:(g + 1) * D] = rows[c]

    in_maps = []
    for b in range(B):
        traj = np.asarray(traj_input[b], dtype=np.int32)   # [256, 3]
        idx6 = np.empty((128, 6), dtype=np.int32)
        idx6[:, 0] = traj[:128, 0]
        idx6[:, 1] = traj[128:, 0]
        idx6[:, 2] = traj[:128, 1]
        idx6[:, 3] = traj[128:, 1]
        idx6[:, 4] = traj[:128, 2]
        idx6[:, 5] = traj[128:, 2]
        in_maps.append({
            "mat": np.ascontiguousarray(
                np.asarray(mat_input[b], dtype=np.float32).reshape(NPAIR, 2)),
            "idx6": idx6,
            "len1": np.array([[traj_length[b]]], dtype=np.int32),
            "wmat": wmat,
            "emb_t": emb_t,
            "emb_l": emb_l,
            "emb_u": emb_u,
        })
    return in_maps


def run(trace=False, **inputs):
    nc = _get_program()
    in_maps = _make_in_maps(**inputs)
    res = run_bass_kernel_spmd(nc, in_maps, core_ids=list(range(B)), trace=trace)
    joint = np.stack([res.results[b]["joint"] for b in range(B)])
    delta = np.stack(
        [res.results[b]["delta"].reshape(L, L, D) for b in range(B)])
    return (joint, delta), res


def kernel(**inputs):
    out, _ = run(trace=False, **inputs)
    return out


# revision 45
# speedup vs baseline: 38239.2184x; 1.0370x over previous
"""Trainium2 Bass kernel for CTRMultiEmbedding (embedding_lookup).

Math (per batch b, with EX_SU=1, EX_SL=0, EX_TU=1, EX_TL=0):
  joint[p,:]   = emb_t[t_idx[p]] + emb_l[l[p]] + emb_u[u[p]]          [256, 64]
  m            = (i < len) & (j < len)                       in {0,1}
  delta[i,j,:] = emb_sl[m]*(1-ds) + emb_su[m]*ds
               + emb_tl[m]*(1-dt) + emb_tu[m]*dt                      [256,256,64]

delta is a per-pair linear combination of 8 raw table rows with scalar
coefficients [(1-m)(1-ds), m(1-ds), (1-m)ds, m ds, (1-m)(1-dt), m(1-dt),
(1-m)dt, m dt].  We compute it as a matmul: for each tile of 1024 pairs,
an 8-block block-diagonal weight matrix W [64, 512] (8 features x 8 pair
blocks; raw table rows on the diagonal blocks) is multiplied by a feature
matrix lhsT [64, 128] so that the PSUM result [128, 512] is *exactly* the
contiguous DRAM layout of 1024 output rows.

Sharding: pure data parallel, batch b -> core b (8 cores, B=8).
"""

import numpy as np

import concourse.bass as bass
import concourse.mybir as mybir
from concourse import bacc
from concourse.bass_utils import run_bass_kernel_spmd
from concourse.masks import make_identity
from concourse.tile import TileContext

B, L, D = 8, 256, 64
NPAIR = L * L            # 65536 pairs per batch
G = 8                    # pair-blocks per matmul tile
NF = 8                   # features per block
K = G * NF               # 64 contraction rows
TILE_PAIRS = 128 * G     # 1024 pairs per tile
NT = NPAIR // TILE_PAIRS  # 64 tiles
NFREE = G * D            # 512 moving columns

F32 = mybir.dt.float32
F32R = mybir.dt.float32r
I32 = mybir.dt.int32


def build_program():
    # Bacc (not raw Bass): its compile step legalizes multi-wait sync into
    # separate wait ops — walrus allows only one sem wait per instruction.
    nc = bacc.Bacc("TRN2", target_bir_lowering=False, debug=False)

    mat = nc.dram_tensor("mat", [NPAIR, 2], F32, kind="ExternalInput")
    idx6 = nc.dram_tensor("idx6", [128, 6], I32, kind="ExternalInput")
    len1 = nc.dram_tensor("len1", [1, 1], I32, kind="ExternalInput")
    wmat = nc.dram_tensor("wmat", [K, NFREE], F32, kind="ExternalInput")
    emb_t = nc.dram_tensor("emb_t", [169, D], F32, kind="ExternalInput")
    emb_l = nc.dram_tensor("emb_l", [50000, D], F32, kind="ExternalInput")
    emb_u = nc.dram_tensor("emb_u", [10000, D], F32, kind="ExternalInput")
    joint = nc.dram_tensor("joint", [L, D], F32, kind="ExternalOutput")
    delta = nc.dram_tensor("delta", [NPAIR, D], F32, kind="ExternalOutput")

    with TileContext(nc) as tc:
        with (
            tc.tile_pool(name="const", bufs=1) as cpool,
            tc.tile_pool(name="big", bufs=1) as bigpool,
            tc.tile_pool(name="lhs", bufs=6) as lhspool,
            tc.tile_pool(name="outp", bufs=8) as outpool,
            tc.tile_pool(name="tpp", bufs=3, space="PSUM") as tppool,
            tc.tile_pool(name="mmp", bufs=5, space="PSUM") as mmpool,
        ):
            # ---------------- one-time setup ----------------
            identity = cpool.tile([128, 128], F32)
            make_identity(nc, identity[:])

            w_f32 = cpool.tile([K, NFREE], F32)
            nc.sync.dma_start(out=w_f32[:], in_=wmat[:])
            # W duplicated in both partition halves: the two matmuls per
            # transposed pair use lhsT at base partition 0 and 64, and
            # matmul requires rhs to share lhsT's base partition.
            w_sb = cpool.tile([128, NFREE], F32R)
            nc.scalar.copy(out=w_sb[0:K, :], in_=w_f32[:])
            nc.scalar.copy(out=w_sb[K:2 * K, :], in_=w_f32[:])

            # mat_all[m, t*16 + g*2 + c] = mat[1024t + 8m + g, c]
            # (loaded in NCHUNK pieces inside the main chunk loop below)
            mat_all = bigpool.tile([128, NT * 16], F32)
            mat_src = mat[:].rearrange("(t m g) c -> m t g c", t=NT, m=128, g=G)

            len_col = cpool.tile([128, 1], I32)
            nc.sync.dma_start(out=len_col[:], in_=len1[:].to_broadcast([128, 1]))
            len_f = cpool.tile([128, 1], F32)
            nc.vector.tensor_copy(out=len_f[:], in_=len_col[:])

            idx_sb = cpool.tile([128, 6], I32)
            nc.sync.dma_start(out=idx_sb[:], in_=idx6[:])

            # ---------------- index / mask precompute ----------------
            # pair q = 1024t + 8m + g;  j = q & 255;  i = 4t + (m >> 5)
            jg = cpool.tile([128, G], I32)
            nc.gpsimd.iota(jg[:], pattern=[[1, G]], base=0, channel_multiplier=8)
            nc.vector.tensor_scalar(
                out=jg[:], in0=jg[:], scalar1=255, scalar2=None,
                op0=mybir.AluOpType.bitwise_and,
            )
            jgf = cpool.tile([128, G], F32)
            nc.vector.tensor_copy(out=jgf[:], in_=jg[:])
            jl = cpool.tile([128, G], F32)  # (j < len) as 1.0/0.0
            nc.vector.tensor_scalar(
                out=jl[:], in0=jgf[:], scalar1=len_f[:, 0:1], scalar2=None,
                op0=mybir.AluOpType.is_lt,
            )

            mcol = cpool.tile([128, 1], I32)
            nc.gpsimd.iota(mcol[:], pattern=[[1, 1]], base=0, channel_multiplier=1)
            nc.vector.tensor_scalar(
                out=mcol[:], in0=mcol[:], scalar1=5, scalar2=None,
                op0=mybir.AluOpType.arith_shift_right,
            )
            mcolf = cpool.tile([128, 1], F32)
            nc.vector.tensor_copy(out=mcolf[:], in_=mcol[:])
            ii = cpool.tile([128, NT], I32)
            nc.gpsimd.iota(ii[:], pattern=[[4, NT]], base=0, channel_multiplier=0)
            iif = cpool.tile([128, NT], F32)
            nc.vector.tensor_copy(out=iif[:], in_=ii[:])
            nc.vector.tensor_scalar(
                out=iif[:], in0=iif[:], scalar1=mcolf[:, 0:1], scalar2=None,
                op0=mybir.AluOpType.add,
            )
            vi = cpool.tile([128, NT], F32)  # (i < len) as 1.0/0.0
            nc.vector.tensor_scalar(
                out=vi[:], in0=iif[:], scalar1=len_f[:, 0:1], scalar2=None,
                op0=mybir.AluOpType.is_lt,
            )

            # Expand vi over g and jl over t to flat [128, NT*G] (t-major)
            # buffers so every big op below is 1-free-dim (S2S2D2 encodings).
            NTG = NT * G
            vix = bigpool.tile([128, NTG], F32)
            nc.gpsimd.tensor_copy(
                out=vix[:],
                in_=vi[:].unsqueeze(2).to_broadcast([128, NT, G]),
            )
            jlx = bigpool.tile([128, NTG], F32)
            nc.gpsimd.tensor_copy(
                out=jlx[:],
                in_=jl[:].unsqueeze(1).to_broadcast([128, NT, G]),
            )
            mask = bigpool.tile([128, NTG], F32)  # vi * jl
            nc.vector.tensor_tensor(
                out=mask[:], in0=vix[:], in1=jlx[:], op=mybir.AluOpType.mult,
            )
            mbar = bigpool.tile([128, NTG], F32)  # 1 - mask
            nc.vector.tensor_scalar(
                out=mbar[:], in0=mask[:], scalar1=-1.0, scalar2=1.0,
                op0=mybir.AluOpType.mult, op1=mybir.AluOpType.add,
            )

            # ---------------- feature matrix F ----------------
            # f_all[m, t*64 + c*8 + g] = feature c of pair (t, m, g)
            # (t-major so each tile's stationary slice is contiguous)
            # Features: c2=mbar*ds c3=mask*ds c6=mbar*dt c7=mask*dt, then
            # c0=mbar-c2, c1=mask-c3, c4=mbar-c6, c5=mask-c7.
            # Built in NCHUNK t-chunks so matmuls can start early.
            f_all = bigpool.tile([128, NF * NTG], F32)
            f6 = f_all[:].rearrange("p (t c g) -> p t c g", t=NT, c=NF, g=G)
            mat4 = mat_all[:].rearrange("p (t g c) -> p t g c", t=NT, g=G, c=2)
            mask3 = mask[:].rearrange("p (t g) -> p t g", t=NT, g=G)
            mbar3 = mbar[:].rearrange("p (t g) -> p t g", t=NT, g=G)
            NCHUNK = 8
            TC = NT // NCHUNK

            delta4 = delta[:].rearrange("(t m g) d -> t m g d", t=NT, m=128, g=G)
            for q in range(NCHUNK):
                ts_ = slice(q * TC, (q + 1) * TC)
                # load this chunk of mat
                nc.sync.dma_start(
                    out=mat4[:, ts_], in_=mat_src[:, ts_],
                )
                msk, mbr = mask3[:, ts_, :], mbar3[:, ts_, :]
                ds = mat4[:, ts_, :, 0]
                dt_ = mat4[:, ts_, :, 1]
                prods = [(2, mbr, ds), (3, msk, ds), (6, mbr, dt_), (7, msk, dt_)]
                for c, a, bb in prods:
                    nc.vector.tensor_tensor(
                        out=f6[:, ts_, c, :], in0=a, in1=bb,
                        op=mybir.AluOpType.mult,
                    )
                diffs = [(0, mbr, 2), (1, msk, 3), (4, mbr, 6), (5, msk, 7)]
                for c, a, csrc in diffs:
                    nc.vector.tensor_tensor(
                        out=f6[:, ts_, c, :], in0=a, in1=f6[:, ts_, csrc, :],
                        op=mybir.AluOpType.subtract,
                    )

                # ---- 16 output tiles of this chunk (2 per transpose) ----
                for tp2 in range(TC // 2):
                    t0 = q * TC + 2 * tp2
                    tp = tppool.tile([128, 128], F32)
                    nc.tensor.transpose(
                        out=tp[:], in_=f_all[:, t0 * K:(t0 + 2) * K],
                        identity=identity[:],
                    )
                    lhsT = lhspool.tile([128, 128], F32R)
                    nc.scalar.copy(out=lhsT[:], in_=tp[:])

                    for half in range(2):
                        t = t0 + half
                        mm = mmpool.tile([128, NFREE], F32)
                        nc.tensor.matmul(
                            out=mm[:], lhsT=lhsT[half * K:(half + 1) * K, :],
                            rhs=w_sb[half * K:(half + 1) * K, :],
                            start=True, stop=True,
                        )

                        ot = outpool.tile([128, NFREE], F32)
                        if t % 16 < 9:
                            nc.scalar.copy(out=ot[:], in_=mm[:])
                        else:
                            nc.vector.tensor_copy(out=ot[:], in_=mm[:])
                        dma_eng = nc.gpsimd if (t % 4 == 3 or t % 16 in (1, 9)) else nc.sync
                        dma_eng.dma_start(
                            out=delta4[t],
                            in_=ot[:].rearrange("p (g d) -> p g d", g=G),
                        )

            # ---------------- joint embedding (gathers) ----------------
            # t_idx = ((t + 167) mod 168) + 1  (t + 167 == t - 1 mod 168, >= 0)
            # No mod ALU op: binary conditional-subtraction in exact f32 ints.
            tf = cpool.tile([128, 2], F32)
            nc.vector.tensor_copy(out=tf[:], in_=idx_sb[:, 4:6])
            nc.vector.tensor_scalar(
                out=tf[:], in0=tf[:], scalar1=167.0, scalar2=None,
                op0=mybir.AluOpType.add,
            )
            tcorr = cpool.tile([128, 2], F32)
            for k in range(5, -1, -1):
                c = float(168 << k)
                # tcorr = (tf >= c) * (-c);  tf += tcorr
                nc.vector.tensor_scalar(
                    out=tcorr[:], in0=tf[:], scalar1=c, scalar2=-c,
                    op0=mybir.AluOpType.is_ge, op1=mybir.AluOpType.mult,
                )
                nc.vector.tensor_tensor(
                    out=tf[:], in0=tf[:], in1=tcorr[:], op=mybir.AluOpType.add,
                )
            nc.vector.tensor_scalar(
                out=tf[:], in0=tf[:], scalar1=1.0, scalar2=None,
                op0=mybir.AluOpType.add,
            )
            tix = cpool.tile([128, 2], I32)
            nc.vector.tensor_copy(out=tix[:], in_=tf[:])

            # Sum the three gathers with DMA inline-accumulate (CCE add):
            # no vector-engine ops needed for the joint embedding at all.
            js = cpool.tile([128, 2 * D], F32)
            gathers = [
                (emb_u, idx_sb[:, 0:1], 0, mybir.AluOpType.bypass),
                (emb_u, idx_sb[:, 1:2], 1, mybir.AluOpType.bypass),
                (emb_l, idx_sb[:, 2:3], 0, mybir.AluOpType.add),
                (emb_l, idx_sb[:, 3:4], 1, mybir.AluOpType.add),
                (emb_t, tix[:, 0:1], 0, mybir.AluOpType.add),
                (emb_t, tix[:, 1:2], 1, mybir.AluOpType.add),
            ]
            for table, iap, half, op in gathers:
                nc.gpsimd.indirect_dma_start(
                    out=js[:, half * D:(half + 1) * D], out_offset=None,
                    in_=table[:],
                    in_offset=bass.IndirectOffsetOnAxis(ap=iap, axis=0),
                    compute_op=op,
                )
            nc.sync.dma_start(
                out=joint[:].rearrange("(h p) d -> p h d", h=2),
                in_=js[:].rearrange("p (h d) -> p h d", h=2),
            )
    nc.finalize()
    return nc


_NC_CACHE = {}


def _get_program():
    if "nc" not in _NC_CACHE:
        _NC_CACHE["nc"] = build_program()
    return _NC_CACHE["nc"]


def _make_in_maps(traj_input, mat_input, traj_length,
                  emb_t, emb_l, emb_u, emb_su, emb_sl, emb_tu, emb_tl):
    emb_t = np.ascontiguousarray(emb_t, dtype=np.float32)
    emb_l = np.ascontiguousarray(emb_l, dtype=np.float32)
    emb_u = np.ascontiguousarray(emb_u, dtype=np.float32)

    # Block-diagonal weights from raw table rows (no arithmetic, placement only).
    rows = [emb_sl[0], emb_sl[1], emb_su[0], emb_su[1],
            emb_tl[0], emb_tl[1], emb_tu[0], emb_tu[1]]
    # lhsT row r = 8c + g (transpose enumerates (c, g) row-major)
    wmat = np.zeros((K, NFREE), dtype=np.float32)
    for g in range(G):
        for c in range(NF):
            wmat[c * G + g, g * D:(g + 1) * D] = rows[c]

    in_maps = []
    for b in range(B):
        traj = np.asarray(traj_input[b], dtype=np.int32)   # [256, 3]
        idx6 = np.empty((128, 6), dtype=np.int32)
        idx6[:, 0] = traj[:128, 0]
        idx6[:, 1] = traj[128:, 0]
        idx6[:, 2] = traj[:128, 1]
        idx6[:, 3] = traj[128:, 1]
        idx6[:, 4] = traj[:128, 2]
        idx6[:, 5] = traj[128:, 2]
        in_maps.append({
            "mat": np.ascontiguousarray(
                np.asarray(mat_input[b], dtype=np.float32).reshape(NPAIR, 2)),
            "idx6": idx6,
            "len1": np.array([[traj_length[b]]], dtype=np.int32),
            "wmat": wmat,
            "emb_t": emb_t,
            "emb_l": emb_l,
            "emb_u": emb_u,
        })
    return in_maps


def run(trace=False, **inputs):
    nc = _get_program()
    in_maps = _make_in_maps(**inputs)
    res = run_bass_kernel_spmd(nc, in_maps, core_ids=list(range(B)), trace=trace)
    joint = np.stack([res.results[b]["joint"] for b in range(B)])
    delta = np.stack(
        [res.results[b]["delta"].reshape(L, L, D) for b in range(B)])
    return (joint, delta), res


def kernel(**inputs):
    out, _ = run(trace=False, **inputs)
    return out
